# revision 20
# baseline (speedup 1.0000x reference)
"""Contrastive loss kernel for 8 TRN2 NeuronCores (Bass/Tile).

Algorithm (host sorts rows by class so same-class pairs are contiguous):
  loss*n = pos + neg
  pos = sum_c cnt_c^2 - sum_c ||v_c||^2       (host, float64 segment sums)
  neg = sum_ij relu(sim-m_i) + sum_i m_i*G_i  minus the same-class part,
        where the same-class part is summed over narrow sorted-class column
        windows (recomputed sim values are bit-identical so the subtraction
        cancels exactly).

Per core: 8 row-chunks x 8 col-chunks of [128,1024] sim tiles (bf16 matmul,
fp32 psum).  The threshold pass (relu with per-row margin, fused row-sum
accumulate) is split ~60/40 between ScalarE (activation) and VectorE
(tensor_scalar cache-reduce); the indicator pass is VectorE is_gt at 4x DVE
mode; margin-weighted counts and the window corrections are reduced on
TensorE via [1|m] weighted matmuls into a persistent PSUM accumulator.
Count matmuls are emitted 2 chunks late so they never head-of-line-block
the PE queue (matmuls complete in strict pc order).
"""

import numpy as np
import ml_dtypes
from contextlib import ExitStack

import concourse.bacc as bacc
import concourse.mybir as mybir
import concourse.tile as tile
from concourse.bass_utils import run_bass_kernel_spmd

N, D, C = 8192, 128, 100
M = 8             # cores
RPC = N // M      # 1024 rows per core
NCH = RPC // 128  # 8 row-chunks per core
CW = 1024         # col-chunk width
NJ = N // CW      # 8 col-chunks
W = 512           # correction window width

BF16 = ml_dtypes.bfloat16

_nc_cache = None
LAST_RESULTS = None


def _dve_relu(idx):
    # ~40% of main chunks run the threshold pass on VectorE
    return idx % 5 in (1, 3)


def _build_nc():
    f32 = mybir.dt.float32
    bf = mybir.dt.bfloat16
    A = mybir.ActivationFunctionType
    OP = mybir.AluOpType

    nc = bacc.Bacc("TRN2", target_bir_lowering=False, debug=False)

    xt = nc.dram_tensor("xt", [128, N], bf, kind="ExternalInput")        # X_sorted^T (full)
    xtl = nc.dram_tensor("xtl", [128, RPC], bf, kind="ExternalInput")    # core's rows, transposed
    xtw = nc.dram_tensor("xtw", [128, NCH * W], bf, kind="ExternalInput")  # correction windows
    mrow = nc.dram_tensor("mrow", [128, NCH], f32, kind="ExternalInput")
    eqm = nc.dram_tensor("eqm", [128, NCH * W], bf, kind="ExternalInput")
    out_acc = nc.dram_tensor("out_acc", [128, 2 * NJ * NCH], f32, kind="ExternalOutput")
    out_red = nc.dram_tensor("out_red", [3, 512], f32, kind="ExternalOutput")

    NCHUNK = NJ * NCH  # 64

    with tile.TileContext(nc) as tc, ExitStack() as ctx:
        consts = ctx.enter_context(tc.tile_pool(name="consts", bufs=1))
        scratch = ctx.enter_context(tc.tile_pool(name="scratch", bufs=3))
        gscratch = ctx.enter_context(tc.tile_pool(name="gscratch", bufs=3))
        wscratch = ctx.enter_context(tc.tile_pool(name="wscratch", bufs=2))
        accp = ctx.enter_context(tc.tile_pool(name="accs", bufs=1))

        dma = nc.default_dma_engine

        xtl_sb = consts.tile([128, RPC], bf)
        dma.dma_start(out=xtl_sb[:], in_=xtl[:])
        m_sb = consts.tile([128, NCH], f32)
        dma.dma_start(out=m_sb[:], in_=mrow[:])
        xt_sb = consts.tile([128, N], bf)
        xtw_sb = consts.tile([128, NCH, W], bf)
        eqm_sb = consts.tile([128, NCH, W], bf)
        for p in range(NJ):
            dma.dma_start(out=xt_sb[:, p * CW:(p + 1) * CW],
                          in_=xt[:, p * CW:(p + 1) * CW])
            if p < NCH:
                dma.dma_start(out=xtw_sb[:, p, :], in_=xtw[:, p * W:(p + 1) * W])
                dma.dma_start(out=eqm_sb[:, p, :], in_=eqm[:, p * W:(p + 1) * W])

        negm = consts.tile([128, NCH], f32)
        nc.vector.tensor_scalar_mul(negm[:], m_sb[:], -1.0)
        neg512m = consts.tile([128, NCH], f32)
        nc.vector.tensor_scalar_mul(neg512m[:], m_sb[:], -512.0)
        m16b = consts.tile([128, NCH], bf)
        nc.vector.tensor_copy(m16b[:], m_sb[:])
        m16f = consts.tile([128, NCH], f32)    # fp32 image of bf16(m)
        nc.vector.tensor_copy(m16f[:], m16b[:])
        onesb = consts.tile([128, 1], bf)
        nc.vector.memset(onesb[:], 1.0)

        oacc = accp.tile([128, 2 * NCHUNK], f32)
        nc.vector.memset(oacc[:], 0.0)

        # work queues for delayed emission (avoid PE head-of-line blocking)
        pend_cnt = []   # (sG tile, ch) -> count matmuls
        pend_win = []   # (jk1, jk2, ch) -> window reduction matmuls
        cnt_started = [False]
        win_started = [[False], [False]]

        with tc.tile_pool(name="ps", bufs=3, space="PSUM") as psum, \
             tc.tile_pool(name="psacc", bufs=1, space="PSUM") as psacc:
            # one psum bank: counts at partitions 0:2, window sums at
            # 32:34 / 64:66 (PE output col-groups are 32-aligned)
            accm = psacc.tile([128, 512], mybir.dt.float32, tag="accm")

            def flush_cnt(keep=0, last=False):
                while len(pend_cnt) > keep:
                    sG_t, ch_t = pend_cnt.pop(0)
                    for q in range(CW // 512):
                        nc.tensor.matmul(accm[0:1, :], onesb[:],
                                         sG_t[:, q * 512:(q + 1) * 512],
                                         start=not cnt_started[0],
                                         stop=last and not pend_cnt and q == CW // 512 - 1,
                                         skip_group_check=True)
                        cnt_started[0] = True

            def flush_win(keep=0, last=False):
                while len(pend_win) > keep:
                    jk1_t, jk2_t, ch_t = pend_win.pop(0)
                    nc.tensor.matmul(accm[32:33, :], onesb[:], jk1_t[:],
                                     start=not win_started[0][0],
                                     stop=last and not pend_win,
                                     skip_group_check=True)
                    win_started[0][0] = True
                    nc.tensor.matmul(accm[64:65, :], onesb[:], jk2_t[:],
                                     start=not win_started[1][0],
                                     stop=last and not pend_win,
                                     skip_group_check=True)
                    win_started[1][0] = True

            for jp in range(NJ // 2):
                for ch in range(NCH):
                  for jj in (2 * jp, 2 * jp + 1):
                    idx = jj * NCH + ch
                    lhsT = xtl_sb[:, ch * 128:(ch + 1) * 128]
                    ps = psum.tile([128, CW], mybir.dt.float32, tag="ps")
                    for q in range(CW // 512):
                        j0 = jj * CW + q * 512
                        nc.tensor.matmul(ps[:, q * 512:(q + 1) * 512], lhsT,
                                         xt_sb[:, j0:j0 + 512],
                                         start=True, stop=True)
                    # tensor_scalar+accum semantics: out = (in0 op0 s1)
                    # elementwise; accum = reduce(out, op1, init=s2), reset
                    # per 512-col psum bank segment.  So: sA = max(ps, m)
                    # (NOT relu!), accum = -512m + sum(max) = sum(relu); the
                    # indicator then thresholds at m instead of 0.
                    sA = scratch.tile([128, CW], bf, tag="sA")
                    if _dve_relu(idx):
                        for q in range(CW // 512):
                            s = slice(q * 512, (q + 1) * 512)
                            nc.vector.tensor_scalar(
                                sA[:, s], ps[:, s], m_sb[:, ch:ch + 1],
                                neg512m[:, ch:ch + 1], OP.max, OP.add,
                                accum_out=oacc[:, 2 * idx + q:2 * idx + q + 1])
                    else:
                        nc.scalar.activation(sA[:], ps[:], A.Relu,
                                             bias=negm[:, ch:ch + 1], scale=1.0,
                                             accum_out=oacc[:, 2 * idx:2 * idx + 1])
                    sG = gscratch.tile([128, CW], bf, tag="sG")
                    thr = m16f[:, ch:ch + 1] if _dve_relu(idx) else 0.0
                    nc.vector.tensor_scalar(sG[:], sA[:], thr,
                                            m16f[:, ch:ch + 1],
                                            OP.is_gt, OP.mult)
                    pend_cnt.append((sG, ch))
                    flush_cnt(keep=2)

                    # same-class window correction: one per row-chunk, spread
                    # across the jj passes
                    if jj == ch:
                        psw = psum.tile([128, CW], mybir.dt.float32, tag="ps")
                        nc.tensor.matmul(psw[:, 0:W], lhsT, xtw_sb[:, ch, :],
                                         start=True, stop=True)
                        uw = wscratch.tile([128, W], bf, tag="uw")
                        nc.scalar.activation(uw[:], psw[:, 0:W], A.Relu,
                                             bias=negm[:, ch:ch + 1], scale=1.0)
                        gw = wscratch.tile([128, W], bf, tag="gw")
                        nc.vector.tensor_scalar(gw[:], uw[:], 0.0,
                                                m16f[:, ch:ch + 1],
                                                OP.is_gt, OP.mult)
                        jk1 = wscratch.tile([128, W], bf, tag="jk1")
                        nc.vector.tensor_mul(jk1[:], eqm_sb[:, ch, :], uw[:])
                        jk2 = wscratch.tile([128, W], bf, tag="jk2")
                        nc.vector.tensor_mul(jk2[:], eqm_sb[:, ch, :], gw[:])
                        pend_win.append((jk1, jk2, ch))
                        flush_win(keep=1)

            flush_cnt(last=True)
            flush_win(last=True)
            red0 = accp.tile([1, 512], f32)
            red1 = accp.tile([1, 512], f32)
            red2 = accp.tile([1, 512], f32)
            nc.scalar.copy(red0[:], accm[0:1, :])
            nc.vector.tensor_copy(red1[:], accm[32:33, :])
            nc.scalar.copy(red2[:], accm[64:65, :])
            dma.dma_start(out=out_red[0:1, :], in_=red0[:])
            dma.dma_start(out=out_red[1:2, :], in_=red1[:])
            dma.dma_start(out=out_red[2:3, :], in_=red2[:])

        dma.dma_start(out=out_acc[:], in_=oacc[:])

    nc.compile()
    return nc


def _prep(inputs, margin, targets):
    """Host-side sharding/layout prep. Returns per-core input maps + class data."""
    t = np.asarray(targets).astype(np.int64)
    x = np.asarray(inputs, dtype=np.float32)
    m = np.asarray(margin, dtype=np.float32)

    perm = np.argsort(t, kind="stable")
    xs, ms, ts = x[perm], m[perm], t[perm]
    x_bf = xs.astype(BF16)
    xt_bf = np.ascontiguousarray(x_bf.T)          # [128, N]

    cnt = np.bincount(ts, minlength=C).astype(np.float64)
    starts = np.concatenate([[0], np.cumsum(cnt).astype(np.int64)])

    # pos term on host: sum_c cnt^2 - sum_c ||sum of class rows||^2 (float64)
    V = np.add.reduceat(xs.astype(np.float64), starts[:-1], axis=0)
    V[cnt == 0] = 0.0
    pos = (cnt ** 2).sum() - (V ** 2).sum()

    nchunks = N // 128
    wstart = np.zeros(nchunks, np.int64)
    for g in range(nchunks):
        lo, hi = ts[g * 128], ts[g * 128 + 127]
        width = starts[hi + 1] - starts[lo]
        assert width <= W - 2, f"class window {width} too wide for chunk {g}"
        wstart[g] = min(int(starts[lo]), N - W) & ~1

    in_maps = []
    for k in range(M):
        r0 = k * RPC
        g0 = r0 // 128
        mr = np.ascontiguousarray(ms[r0:r0 + RPC].reshape(NCH, 128).T)
        xtw_ = np.concatenate(
            [xt_bf[:, wstart[g0 + ch]:wstart[g0 + ch] + W] for ch in range(NCH)], axis=1)
        eqm_ = np.concatenate(
            [(ts[r0 + ch * 128:r0 + (ch + 1) * 128, None]
              == ts[None, wstart[g0 + ch]:wstart[g0 + ch] + W]).astype(BF16)
             for ch in range(NCH)], axis=1)
        in_maps.append({
            "xt": xt_bf,
            "xtl": np.ascontiguousarray(xt_bf[:, r0:r0 + RPC]),
            "xtw": np.ascontiguousarray(xtw_),
            "mrow": mr,
            "eqm": np.ascontiguousarray(eqm_),
        })
    return in_maps, pos


def kernel(inputs, margin, targets):
    global _nc_cache, LAST_RESULTS
    in_maps, pos = _prep(inputs, margin, targets)
    if _nc_cache is None:
        _nc_cache = _build_nc()
    res = run_bass_kernel_spmd(_nc_cache, in_maps, list(range(M)))
    LAST_RESULTS = res

    neg = 0.0
    for k in range(M):
        r = res.results[k]
        red = r["out_red"].astype(np.float64)
        neg += r["out_acc"].astype(np.float64).sum()   # sum relu(sim - m)
        neg += red[0].sum()                            # sum m16 * [sim > m]
        neg -= red[1].sum()                            # same-class relu corr
        neg -= red[2].sum()                            # same-class m16*cnt corr

    loss = (pos + neg) / N
    return np.float32(loss)


# revision 21
# speedup vs baseline: 1.0612x; 1.0612x over previous
"""Contrastive loss kernel for 8 TRN2 NeuronCores (Bass/Tile).

Algorithm (host sorts rows by class so same-class pairs are contiguous):
  loss*n = pos + neg
  pos = sum_c cnt_c^2 - sum_c ||v_c||^2       (host, float64 segment sums)
  neg = sum_ij relu(sim-m_i) + sum_i m_i*G_i  minus the same-class part,
        where the same-class part is summed over narrow sorted-class column
        windows (recomputed sim values are bit-identical so the subtraction
        cancels exactly).

Per core: 8 row-chunks x 8 col-chunks of [128,1024] sim tiles (bf16 matmul,
fp32 psum).  The threshold pass (relu with per-row margin, fused row-sum
accumulate) is split ~60/40 between ScalarE (activation) and VectorE
(tensor_scalar cache-reduce); the indicator pass is VectorE is_gt at 4x DVE
mode; margin-weighted counts and the window corrections are reduced on
TensorE via [1|m] weighted matmuls into a persistent PSUM accumulator.
Count matmuls are emitted 2 chunks late so they never head-of-line-block
the PE queue (matmuls complete in strict pc order).
"""

import numpy as np
import ml_dtypes
from contextlib import ExitStack

import concourse.bacc as bacc
import concourse.mybir as mybir
import concourse.tile as tile
from concourse.bass_utils import run_bass_kernel_spmd

N, D, C = 8192, 128, 100
M = 8             # cores
RPC = N // M      # 1024 rows per core
NCH = RPC // 128  # 8 row-chunks per core
CW = 1024         # col-chunk width
NJ = N // CW      # 8 col-chunks
W = 512           # correction window width

BF16 = ml_dtypes.bfloat16

_nc_cache = None
LAST_RESULTS = None


def _dve_relu(idx):
    # ~40% of main chunks run the threshold pass on VectorE
    return idx % 5 in (1, 3)


def _build_nc():
    f32 = mybir.dt.float32
    bf = mybir.dt.bfloat16
    A = mybir.ActivationFunctionType
    OP = mybir.AluOpType

    nc = bacc.Bacc("TRN2", target_bir_lowering=False, debug=False)

    xt = nc.dram_tensor("xt", [128, N], bf, kind="ExternalInput")        # X_sorted^T (full)
    xtl = nc.dram_tensor("xtl", [128, RPC], bf, kind="ExternalInput")    # core's rows, transposed
    xtw = nc.dram_tensor("xtw", [128, NCH * W], bf, kind="ExternalInput")  # correction windows
    mrow = nc.dram_tensor("mrow", [128, NCH], f32, kind="ExternalInput")
    eqm = nc.dram_tensor("eqm", [128, NCH * W], bf, kind="ExternalInput")
    out_acc = nc.dram_tensor("out_acc", [128, 2 * NJ * NCH], f32, kind="ExternalOutput")
    out_red = nc.dram_tensor("out_red", [3, 512], f32, kind="ExternalOutput")

    NCHUNK = NJ * NCH  # 64

    with tile.TileContext(nc) as tc, ExitStack() as ctx:
        consts = ctx.enter_context(tc.tile_pool(name="consts", bufs=1))
        scratch = ctx.enter_context(tc.tile_pool(name="scratch", bufs=3))
        gscratch = ctx.enter_context(tc.tile_pool(name="gscratch", bufs=3))
        wscratch = ctx.enter_context(tc.tile_pool(name="wscratch", bufs=2))
        accp = ctx.enter_context(tc.tile_pool(name="accs", bufs=1))

        dma = nc.default_dma_engine

        xtl_sb = consts.tile([128, RPC], bf)
        dma.dma_start(out=xtl_sb[:], in_=xtl[:])
        m_sb = consts.tile([128, NCH], f32)
        dma.dma_start(out=m_sb[:], in_=mrow[:])
        xt_sb = consts.tile([128, N], bf)
        xtw_sb = consts.tile([128, NCH, W], bf)
        eqm_sb = consts.tile([128, NCH, W], bf)
        for p in range(NJ):
            dma.dma_start(out=xt_sb[:, p * CW:(p + 1) * CW],
                          in_=xt[:, p * CW:(p + 1) * CW])
            if p < NCH:
                dma.dma_start(out=xtw_sb[:, p, :], in_=xtw[:, p * W:(p + 1) * W])
                dma.dma_start(out=eqm_sb[:, p, :], in_=eqm[:, p * W:(p + 1) * W])

        negm = consts.tile([128, NCH], f32)
        nc.vector.tensor_scalar_mul(negm[:], m_sb[:], -1.0)
        neg512m = consts.tile([128, NCH], f32)
        nc.vector.tensor_scalar_mul(neg512m[:], m_sb[:], -512.0)
        m16b = consts.tile([128, NCH], bf)
        nc.vector.tensor_copy(m16b[:], m_sb[:])
        m16f = consts.tile([128, NCH], f32)    # fp32 image of bf16(m)
        nc.vector.tensor_copy(m16f[:], m16b[:])
        onesb = consts.tile([128, 1], bf)
        nc.vector.memset(onesb[:], 1.0)

        oacc = accp.tile([128, 2 * NCHUNK], f32)
        nc.vector.memset(oacc[:], 0.0)

        # work queues for delayed emission (avoid PE head-of-line blocking)
        pend_cnt = []   # (sG tile, ch) -> count matmuls
        pend_win = []   # (jk1, jk2, ch) -> window reduction matmuls
        cnt_started = [False]
        win_started = [[False], [False]]

        with tc.tile_pool(name="ps", bufs=3, space="PSUM") as psum, \
             tc.tile_pool(name="psacc", bufs=1, space="PSUM") as psacc:
            # one psum bank: counts at partitions 0:2, window sums at
            # 32:34 / 64:66 (PE output col-groups are 32-aligned)
            accm = psacc.tile([128, 512], mybir.dt.float32, tag="accm")

            def flush_cnt(keep=0, last=False):
                while len(pend_cnt) > keep:
                    sG_t, ch_t = pend_cnt.pop(0)
                    for q in range(CW // 512):
                        nc.tensor.matmul(accm[0:1, :], onesb[:],
                                         sG_t[:, q * 512:(q + 1) * 512],
                                         start=not cnt_started[0],
                                         stop=last and not pend_cnt and q == CW // 512 - 1,
                                         skip_group_check=True)
                        cnt_started[0] = True

            def flush_win(keep=0, last=False):
                while len(pend_win) > keep:
                    jk1_t, jk2_t, ch_t = pend_win.pop(0)
                    nc.tensor.matmul(accm[32:33, :], onesb[:], jk1_t[:],
                                     start=not win_started[0][0],
                                     stop=last and not pend_win,
                                     skip_group_check=True)
                    win_started[0][0] = True
                    nc.tensor.matmul(accm[64:65, :], onesb[:], jk2_t[:],
                                     start=not win_started[1][0],
                                     stop=last and not pend_win,
                                     skip_group_check=True)
                    win_started[1][0] = True

            for jj in range(NJ):
                for ch in range(NCH):
                    idx = jj * NCH + ch
                    lhsT = xtl_sb[:, ch * 128:(ch + 1) * 128]
                    ps = psum.tile([128, CW], mybir.dt.float32, tag="ps")
                    for q in range(CW // 512):
                        j0 = jj * CW + q * 512
                        nc.tensor.matmul(ps[:, q * 512:(q + 1) * 512], lhsT,
                                         xt_sb[:, j0:j0 + 512],
                                         start=True, stop=True)
                    # tensor_scalar+accum semantics: out = (in0 op0 s1)
                    # elementwise; accum = reduce(out, op1, init=s2), reset
                    # per 512-col psum bank segment.  So: sA = max(ps, m)
                    # (NOT relu!), accum = -512m + sum(max) = sum(relu); the
                    # indicator then thresholds at m instead of 0.
                    sA = scratch.tile([128, CW], bf, tag="sA")
                    if _dve_relu(idx):
                        for q in range(CW // 512):
                            s = slice(q * 512, (q + 1) * 512)
                            nc.vector.tensor_scalar(
                                sA[:, s], ps[:, s], m_sb[:, ch:ch + 1],
                                neg512m[:, ch:ch + 1], OP.max, OP.add,
                                accum_out=oacc[:, 2 * idx + q:2 * idx + q + 1])
                    else:
                        nc.scalar.activation(sA[:], ps[:], A.Relu,
                                             bias=negm[:, ch:ch + 1], scale=1.0,
                                             accum_out=oacc[:, 2 * idx:2 * idx + 1])
                    sG = gscratch.tile([128, CW], bf, tag="sG")
                    thr = m16f[:, ch:ch + 1] if _dve_relu(idx) else 0.0
                    nc.vector.tensor_scalar(sG[:], sA[:], thr,
                                            m16f[:, ch:ch + 1],
                                            OP.is_gt, OP.mult)
                    pend_cnt.append((sG, ch))
                    flush_cnt(keep=2)

                    # same-class window correction: one per row-chunk, spread
                    # across the jj passes
                    if jj == ch:
                        psw = psum.tile([128, CW], mybir.dt.float32, tag="ps")
                        nc.tensor.matmul(psw[:, 0:W], lhsT, xtw_sb[:, ch, :],
                                         start=True, stop=True)
                        uw = wscratch.tile([128, W], bf, tag="uw")
                        nc.scalar.activation(uw[:], psw[:, 0:W], A.Relu,
                                             bias=negm[:, ch:ch + 1], scale=1.0)
                        gw = wscratch.tile([128, W], bf, tag="gw")
                        nc.vector.tensor_scalar(gw[:], uw[:], 0.0,
                                                m16f[:, ch:ch + 1],
                                                OP.is_gt, OP.mult)
                        jk1 = wscratch.tile([128, W], bf, tag="jk1")
                        nc.vector.tensor_mul(jk1[:], eqm_sb[:, ch, :], uw[:])
                        jk2 = wscratch.tile([128, W], bf, tag="jk2")
                        nc.vector.tensor_mul(jk2[:], eqm_sb[:, ch, :], gw[:])
                        pend_win.append((jk1, jk2, ch))
                        flush_win(keep=1)

            flush_cnt(last=True)
            flush_win(last=True)
            red0 = accp.tile([1, 512], f32)
            red1 = accp.tile([1, 512], f32)
            red2 = accp.tile([1, 512], f32)
            nc.scalar.copy(red0[:], accm[0:1, :])
            nc.vector.tensor_copy(red1[:], accm[32:33, :])
            nc.scalar.copy(red2[:], accm[64:65, :])
            dma.dma_start(out=out_red[0:1, :], in_=red0[:])
            dma.dma_start(out=out_red[1:2, :], in_=red1[:])
            dma.dma_start(out=out_red[2:3, :], in_=red2[:])

        dma.dma_start(out=out_acc[:], in_=oacc[:])

    nc.compile()
    return nc


def _prep(inputs, margin, targets):
    """Host-side sharding/layout prep. Returns per-core input maps + class data."""
    t = np.asarray(targets).astype(np.int64)
    x = np.asarray(inputs, dtype=np.float32)
    m = np.asarray(margin, dtype=np.float32)

    perm = np.argsort(t, kind="stable")
    xs, ms, ts = x[perm], m[perm], t[perm]
    x_bf = xs.astype(BF16)
    xt_bf = np.ascontiguousarray(x_bf.T)          # [128, N]

    cnt = np.bincount(ts, minlength=C).astype(np.float64)
    starts = np.concatenate([[0], np.cumsum(cnt).astype(np.int64)])

    # pos term on host: sum_c cnt^2 - sum_c ||sum of class rows||^2 (float64)
    V = np.add.reduceat(xs.astype(np.float64), starts[:-1], axis=0)
    V[cnt == 0] = 0.0
    pos = (cnt ** 2).sum() - (V ** 2).sum()

    nchunks = N // 128
    wstart = np.zeros(nchunks, np.int64)
    for g in range(nchunks):
        lo, hi = ts[g * 128], ts[g * 128 + 127]
        width = starts[hi + 1] - starts[lo]
        assert width <= W - 2, f"class window {width} too wide for chunk {g}"
        wstart[g] = min(int(starts[lo]), N - W) & ~1

    in_maps = []
    for k in range(M):
        r0 = k * RPC
        g0 = r0 // 128
        mr = np.ascontiguousarray(ms[r0:r0 + RPC].reshape(NCH, 128).T)
        xtw_ = np.concatenate(
            [xt_bf[:, wstart[g0 + ch]:wstart[g0 + ch] + W] for ch in range(NCH)], axis=1)
        eqm_ = np.concatenate(
            [(ts[r0 + ch * 128:r0 + (ch + 1) * 128, None]
              == ts[None, wstart[g0 + ch]:wstart[g0 + ch] + W]).astype(BF16)
             for ch in range(NCH)], axis=1)
        in_maps.append({
            "xt": xt_bf,
            "xtl": np.ascontiguousarray(xt_bf[:, r0:r0 + RPC]),
            "xtw": np.ascontiguousarray(xtw_),
            "mrow": mr,
            "eqm": np.ascontiguousarray(eqm_),
        })
    return in_maps, pos


def kernel(inputs, margin, targets):
    global _nc_cache, LAST_RESULTS
    in_maps, pos = _prep(inputs, margin, targets)
    if _nc_cache is None:
        _nc_cache = _build_nc()
    res = run_bass_kernel_spmd(_nc_cache, in_maps, list(range(M)))
    LAST_RESULTS = res

    neg = 0.0
    for k in range(M):
        r = res.results[k]
        red = r["out_red"].astype(np.float64)
        neg += r["out_acc"].astype(np.float64).sum()   # sum relu(sim - m)
        neg += red[0].sum()                            # sum m16 * [sim > m]
        neg -= red[1].sum()                            # same-class relu corr
        neg -= red[2].sum()                            # same-class m16*cnt corr

    loss = (pos + neg) / N
    return np.float32(loss)
